# revision 38
# baseline (speedup 1.0000x reference)
"""Trainium2 Bass kernel: multi-head attention (B=4, T=2048, D=2048, H=16).

Sharding: 8 cores = 4 batches x 2 head-groups (tensor-parallel heads, data-
parallel batch). Each core handles one batch and 8 heads (f-slice of 1024
columns of the QKV projections / rows of the out-projection). Host sums the
two partial out-projection results per batch and adds the output bias
(with bv folded in: softmax rows sum to 1, so the v-bias contributes the
constant bv @ Wo.T to every output row).

Per-core pipeline:
  Phase V: v[t,f] via x-stationary bf16 matmuls, two half-F subpasses.
  Phase QKA (per head): q/k projections in fp8 e4m3 DoubleRow mode (2
           k-tiles per matmul = 256-deep contraction at 2x PE rate; W
           pre-scaled x256 on host to avoid e4m3 subnormals, exp scale
           divided by 65536 to compensate -- softmax is scale-invariant).
           Attention per 512-wide q-stage: S^T[k,q] on PE, exp on ScalarE
           (stage rate-limiter), PV transposed (v chunks stationary, P^T
           moving, per-kc deps let PE trail the exp stream); softmax
           denominator via DVE bf16 pair-add tree (16 chunks -> 2) + one
           accumulated all-ones matmul (partition broadcast), then
           reciprocal_approx_fast and a single DVE multiply.
  Phase C: out^T[d,t] = WoT.T @ yT accumulated over f-chunks; stored
           transposed, host transposes back.
"""

import sys

if "/opt/trn_rl_repo" not in sys.path:
    sys.path.insert(0, "/opt/trn_rl_repo")

import numpy as np
import ml_dtypes

D = 2048          # d_model
T = 2048          # sequence length
B = 4             # batch
H = 16            # total heads
DH = 128          # head dim
GROUPS = 2        # head groups (tensor-parallel factor per batch)
HG = H // GROUPS  # heads per core = 8
F = HG * DH       # per-core projection width = 1024
P = 128
DC = D // P       # 16 contraction chunks
TC = T // P       # 16 t chunks
NCORES = 8
QS = 512          # attention stage width (q columns per stage)
NST = T // QS     # 4 stages per head
WSCALE = 256.0    # host pre-scale on Wq/Wk before e4m3 quantization
SCALE = float(1.0 / np.sqrt(DH))
EXP_SCALE = SCALE / (WSCALE * WSCALE)

_PROGRAM = None


def _build_program():
    import concourse.bass as bass
    import concourse.tile as tile
    from concourse import bacc, mybir
    from concourse.bass import ts, ds

    bf16 = mybir.dt.bfloat16
    f32 = mybir.dt.float32
    f8 = mybir.dt.float8e4

    nc = bacc.Bacc("TRN2", target_bir_lowering=False, debug=False,
                   num_devices=NCORES)

    x8_d = nc.dram_tensor("x8", [DC, P, T], f8, kind="ExternalInput")
    xT_d = nc.dram_tensor("xT", [DC, P, T], bf16, kind="ExternalInput")
    wq8_d = nc.dram_tensor("wq8", [HG, P, DC * DH], f8, kind="ExternalInput")
    wk8_d = nc.dram_tensor("wk8", [HG, P, DC * DH], f8, kind="ExternalInput")
    wv_d = nc.dram_tensor("wv", [DC, P, F], bf16, kind="ExternalInput")
    wo_d = nc.dram_tensor("wo", [DC, P, HG, P], bf16, kind="ExternalInput")
    bq_d = nc.dram_tensor("bq", [P, HG], f32, kind="ExternalInput")
    bk_d = nc.dram_tensor("bk", [P, HG], f32, kind="ExternalInput")
    out_d = nc.dram_tensor("out", [DC, P, T], f32, kind="ExternalOutput")

    Exp = mybir.ActivationFunctionType.Exp
    DR = mybir.MatmulPerfMode.DoubleRow
    MULT = mybir.AluOpType.mult

    with tile.TileContext(nc) as tc:
        from contextlib import ExitStack
        with ExitStack() as ctx:
            # ---- persistent pools (live whole kernel) ----
            const = ctx.enter_context(tc.tile_pool(name="const", bufs=1))
            vpool = ctx.enter_context(tc.tile_pool(name="vpool", bufs=1))
            ytp = ctx.enter_context(tc.tile_pool(name="ytp", bufs=1))
            x8p = ctx.enter_context(tc.tile_pool(name="x8p", bufs=1))
            w8p = ctx.enter_context(tc.tile_pool(name="w8p", bufs=2))
            qkt = ctx.enter_context(tc.tile_pool(name="qkt", bufs=2))
            wop = ctx.enter_context(tc.tile_pool(name="wop", bufs=4))
            ps_qk = ctx.enter_context(
                tc.tile_pool(name="ps_qk", bufs=2, space="PSUM"))

            zero_b = const.tile([P, 1], f32, tag="zerob")
            nc.vector.memset(zero_b[:], 0.0)
            ones_sb = const.tile([P, P], bf16, tag="ones")
            nc.vector.memset(ones_sb[:], 1.0)
            bq_sb = const.tile([P, HG], f32, tag="bq")
            bk_sb = const.tile([P, HG], f32, tag="bk")

            v_sb = vpool.tile([P, TC, HG, DH], bf16, tag="v")
            yT = ytp.tile([P, HG, T], bf16, tag="yT")
            x8 = x8p.tile([P, DC, T], f8, tag="x8")

            # force early allocation of persistent pools (first-use order)
            nc.vector.memset(v_sb[:, 0, 0, 0:1], 0.0)
            nc.vector.memset(yT[:, 0, 0:1], 0.0)

            w8q = [w8p.tile([P, DC, DH], f8, tag="w8q", name=f"w8q{h}")
                   for h in range(HG)]
            w8k = [w8p.tile([P, DC, DH], f8, tag="w8k", name=f"w8k{h}")
                   for h in range(HG)]

            def dma_w8(h):
                nc.sync.dma_start(
                    w8q[h][:],
                    wq8_d[h].rearrange("p (c f) -> p c f", f=DH))
                nc.sync.dma_start(
                    w8k[h][:],
                    wk8_d[h].rearrange("p (c f) -> p c f", f=DH))

            qts = {}

            def pg_open(h, kind, tb):
                if kind == "k":
                    w8h, bias_sb = w8k[h], bk_sb
                    if tb == 0:
                        qts.setdefault(h, {})["kT"] = qkt.tile(
                            [P, T], bf16, tag="kT", name=f"kT{h}")
                    dst = qts[h]["kT"]
                else:
                    w8h, bias_sb = w8q[h], bq_sb
                    if tb == 0:
                        qts.setdefault(h, {})["qT"] = qkt.tile(
                            [P, T], bf16, tag="qT", name=f"qT{h}")
                    dst = qts[h]["qT"]
                ps = ps_qk.tile([P, QS], f32, tag="psqk",
                                name=f"ps{kind}{h}_{tb}")
                return [h, kind, tb, w8h, bias_sb, dst, ps, 0]

            def pg_step(pg):
                h, kind, tb, w8h, bias_sb, dst, ps, d = pg
                nc.tensor.matmul(
                    ps[:],
                    w8h[:, ds(2 * d, 2), :],
                    x8[:, ds(2 * d, 2), ds(tb * QS, QS)],
                    start=(d == 0), stop=(d == DC // 2 - 1),
                    perf_mode=DR)
                pg[7] = d + 1
                if pg[7] == DC // 2:
                    nc.vector.tensor_scalar_add(
                        dst[:, ds(tb * QS, QS)], ps[:],
                        bias_sb[:, ds(h, 1)])
                    return None
                return pg

            def proj_group(h, kind, tb):
                pg = pg_open(h, kind, tb)
                while pg is not None:
                    pg = pg_step(pg)

            prologue = [(0, "k", 0), (0, "k", 1), (0, "k", 2), (0, "k", 3),
                        (0, "q", 0)]

            # ---------------- Phase V: v projection ----------------
            with tc.tile_pool(name="wvp", bufs=1) as wvp, \
                 tc.tile_pool(name="xbv", bufs=3) as xbv, \
                 tc.tile_pool(name="ps_v", bufs=6, space="PSUM") as ps_v:
                for half in range(2):
                    f0 = half * 512
                    wvh = wvp.tile([P, DC, 512], bf16, tag="wvh",
                                   name=f"wvh{half}")
                    xbs = [xbv.tile([P, DC, 512], bf16, tag="xbv",
                                    name=f"xbv{half}_{tcb}")
                           for tcb in range(4)]
                    if half == 0:
                        # single-dc first pair so dc 0 lands fast
                        nc.sync.dma_start(
                            wvh[:, ds(0, 1)],
                            wv_d[ds(0, 1), :, ds(f0, 512)].rearrange(
                                "c p f -> p c f"))
                        nc.sync.dma_start(
                            xbs[0][:, ds(0, 1)],
                            xT_d[ds(0, 1), :, ds(0, 512)].rearrange(
                                "c p t -> p c t"))
                        nc.sync.dma_start(
                            wvh[:, ds(1, 3)],
                            wv_d[ds(1, 3), :, ds(f0, 512)].rearrange(
                                "c p f -> p c f"))
                        nc.sync.dma_start(
                            xbs[0][:, ds(1, 3)],
                            xT_d[ds(1, 3), :, ds(0, 512)].rearrange(
                                "c p t -> p c t"))
                        for k in range(1, 4):
                            nc.sync.dma_start(
                                wvh[:, ds(4 * k, 4)],
                                wv_d[ds(4 * k, 4), :, ds(f0, 512)].rearrange(
                                    "c p f -> p c f"))
                            nc.sync.dma_start(
                                xbs[0][:, ds(4 * k, 4)],
                                xT_d[ds(4 * k, 4), :, ds(0, 512)].rearrange(
                                    "c p t -> p c t"))
                        for tcb in range(1, 4):
                            nc.sync.dma_start(
                                xbs[tcb][:],
                                xT_d[:, :, ds(tcb * 512, 512)].rearrange(
                                    "c p t -> p c t"))
                        # x8 + head-0/1 fp8 weights, behind the v-pass loads
                        nc.sync.dma_start(bq_sb[:], bq_d[:])
                        nc.sync.dma_start(bk_sb[:], bk_d[:])
                        nc.sync.dma_start(
                            x8[:], x8_d.rearrange("c p t -> p c t"))
                        dma_w8(0)
                        dma_w8(1)
                    else:
                        for k in range(4):
                            nc.sync.dma_start(
                                wvh[:, ds(4 * k, 4)],
                                wv_d[ds(4 * k, 4), :, ds(f0, 512)].rearrange(
                                    "c p f -> p c f"))
                        for tcb in range(4):
                            nc.sync.dma_start(
                                xbs[tcb][:],
                                xT_d[:, :, ds(tcb * 512, 512)].rearrange(
                                    "c p t -> p c t"))
                    for tcb in range(4):
                        xb = xbs[tcb]
                        for tsub in range(4):
                            tc_ = tcb * 4 + tsub
                            ps = ps_v.tile([P, 512], f32, tag="psv",
                                           name=f"psv{half}_{tc_}")
                            for dc in range(DC):
                                nc.tensor.matmul(
                                    ps[:],
                                    xb[:, dc, ds(tsub * P, P)],
                                    wvh[:, dc],
                                    start=(dc == 0), stop=(dc == DC - 1))
                            nc.vector.tensor_copy(
                                v_sb[:, tc_, ds(half * 4, 4), :],
                                ps[:].rearrange("p (h d) -> p h d", d=DH))
                            if half == 1 and tcb >= 2 and prologue:
                                proj_group(*prologue.pop(0))

            # ---------------- Phase QKA: q/k proj + attention ----------
            with tc.tile_pool(name="ptp", bufs=3) as ptp, \
                 tc.tile_pool(name="t8p", bufs=1) as t8p, \
                 tc.tile_pool(name="t4p", bufs=1) as t4p, \
                 tc.tile_pool(name="t2p", bufs=1) as t2p, \
                 tc.tile_pool(name="t1p", bufs=1) as t1p, \
                 tc.tile_pool(name="rinvp", bufs=2) as rinvp, \
                 tc.tile_pool(name="ps_st", bufs=3, space="PSUM") as ps_st, \
                 tc.tile_pool(name="ps_pv", bufs=2, space="PSUM") as ps_pv, \
                 tc.tile_pool(name="ps_bc", bufs=1, space="PSUM") as ps_bc:

                def finish(rec):
                    # softmax denominator broadcast + normalize (deferred
                    # one stage so the PE never waits on the DVE add-tree)
                    h, s, yps, t1 = rec
                    bc = ps_bc.tile([P, QS], f32, tag="bc", name=f"bc{h}_{s}")
                    nc.tensor.matmul(bc[:], ones_sb[:], t1[:],
                                     start=True, stop=True)
                    rinv = rinvp.tile([P, QS], f32, tag="rinv",
                                      name=f"rinv{h}_{s}")
                    nc.vector.reciprocal_approx_fast(rinv[:], bc[:])
                    nc.vector.tensor_tensor(
                        yT[:, h, ds(s * QS, QS)], yps[:], rinv[:], MULT)

                while prologue:
                    proj_group(*prologue.pop(0))
                workq = []
                cur_pg = None
                prev = None  # previous stage: (h, s, pt_t, yps, t1)
                wo_tiles = {}

                def wo_fetch(dch):
                    wo_tiles[dch] = wop.tile([P, HG, P], bf16, tag="wo",
                                             name=f"wo{dch}")
                    nc.sync.dma_start(wo_tiles[dch][:], wo_d[dch])

                for g in range(HG * NST):
                    h, s = divmod(g, NST)
                    if g >= HG * NST - 4:
                        wo_fetch(g - (HG * NST - 4))
                    if s == 0:
                        if h + 2 < HG:
                            dma_w8(h + 2)
                        for tb in range(1, NST):
                            workq.append((h, "q", tb))
                        if h + 1 < HG:
                            for tb in range(NST):
                                workq.append((h + 1, "k", tb))
                            workq.append((h + 1, "q", 0))
                    kT, qT = qts[h]["kT"], qts[h]["qT"]
                    # S^T[k,q] per k-chunk (exp -> P^T chunk), with the
                    # PREVIOUS stage's PV matmuls woven in chunk-by-chunk:
                    # they are always ready, so the PE never drains while
                    # ScalarE grinds through this stage's exp stream
                    pt_t = ptp.tile([P, TC, QS], bf16, tag="pt",
                                    name=f"pt{h}_{s}")
                    yps_prev = None
                    for kc in range(TC):
                        st = ps_st.tile([P, QS], f32, tag="st",
                                        name=f"st{h}{s}{kc}")
                        nc.tensor.matmul(st[:], kT[:, ds(kc * P, P)],
                                         qT[:, ds(s * QS, QS)],
                                         start=True, stop=True)
                        nc.scalar.activation(pt_t[:, kc], st[:], Exp,
                                             bias=0.0, scale=EXP_SCALE)
                        # previous stage's PV woven in chunk-by-chunk: its
                        # operands are always ready, so the PE never drains
                        # while ScalarE grinds through this stage's exps
                        if prev is not None:
                            if yps_prev is None:
                                yps_prev = ps_pv.tile(
                                    [P, QS], f32, tag="yps",
                                    name=f"yps{prev[0]}_{prev[1]}")
                            nc.tensor.matmul(yps_prev[:],
                                             v_sb[:, kc, prev[0]],
                                             prev[2][:, kc],
                                             start=(kc == 0),
                                             stop=(kc == TC - 1))
                    # rowsum: bf16 pair-add tree 16 -> 1 chunk on DVE
                    t8 = t8p.tile([P, 8, QS], bf16, tag="t8",
                                  name=f"t8_{h}_{s}")
                    for i in range(8):
                        nc.vector.tensor_add(t8[:, i], pt_t[:, 2 * i],
                                             pt_t[:, 2 * i + 1])
                    t4 = t4p.tile([P, 4, QS], bf16, tag="t4",
                                  name=f"t4_{h}_{s}")
                    for i in range(4):
                        nc.vector.tensor_add(t4[:, i], t8[:, 2 * i],
                                             t8[:, 2 * i + 1])
                    t2 = t2p.tile([P, 2, QS], bf16, tag="t2",
                                  name=f"t2_{h}_{s}")
                    for i in range(2):
                        nc.vector.tensor_add(t2[:, i], t4[:, 2 * i],
                                             t4[:, 2 * i + 1])
                    t1 = t1p.tile([P, QS], bf16, tag="t1",
                                  name=f"t1_{h}_{s}")
                    nc.vector.tensor_add(t1[:], t2[:, 0], t2[:, 1])
                    if prev is not None:
                        finish((prev[0], prev[1], yps_prev, prev[4]))
                    # projection groups for upcoming stages: anti-phase
                    # block that fills PE while this stage's exps drain
                    for _ in range(2):
                        if workq:
                            proj_group(*workq.pop(0))
                    prev = (h, s, pt_t, None, t1)
                # epilogue: PV + normalize for the final stage
                h, s, pt_t, _, t1 = prev
                yps = ps_pv.tile([P, QS], f32, tag="yps", name=f"yps{h}_{s}")
                for kc in range(TC):
                    nc.tensor.matmul(yps[:], v_sb[:, kc, h], pt_t[:, kc],
                                     start=(kc == 0), stop=(kc == TC - 1))
                finish((h, s, yps, t1))

            # ---------------- Phase C: out-projection ----------------
            with tc.tile_pool(name="osb", bufs=8) as osb, \
                 tc.tile_pool(name="ps_o", bufs=6, space="PSUM") as ps_o:
                for dch in range(DC):
                    if dch in wo_tiles:
                        wo_t = wo_tiles[dch]
                    else:
                        wo_t = wop.tile([P, HG, P], bf16, tag="wo",
                                        name=f"wo{dch}")
                        nc.sync.dma_start(wo_t[:], wo_d[dch])
                    pso = [ps_o.tile([P, 512], f32, tag="pso",
                                     name=f"pso{dch}_{i}") for i in range(4)]
                    if dch < DC - 1:
                        for fc in range(HG):
                            for tcb in range(4):
                                nc.tensor.matmul(
                                    pso[tcb][:],
                                    wo_t[:, fc],
                                    yT[:, fc, ds(tcb * 512, 512)],
                                    start=(fc == 0), stop=(fc == HG - 1))
                        for tcb in range(4):
                            ot = osb.tile([P, 512], f32, tag="ot",
                                          name=f"ot{dch}_{tcb}")
                            nc.vector.tensor_copy(ot[:], pso[tcb][:])
                            nc.sync.dma_start(
                                out_d[dch, :, ds(tcb * 512, 512)], ot[:])
                    else:
                        # last dch tcb-major: each psum finishes separately
                        # so 3 of 4 copy+DMA chains overlap remaining matmuls
                        for tcb in range(4):
                            for fc in range(HG):
                                nc.tensor.matmul(
                                    pso[tcb][:],
                                    wo_t[:, fc],
                                    yT[:, fc, ds(tcb * 512, 512)],
                                    start=(fc == 0), stop=(fc == HG - 1))
                            ot = osb.tile([P, 512], f32, tag="ot",
                                          name=f"ot{dch}_{tcb}")
                            nc.vector.tensor_copy(ot[:], pso[tcb][:])
                            nc.sync.dma_start(
                                out_d[dch, :, ds(tcb * 512, 512)], ot[:])

    nc.compile()
    return nc


def _get_program():
    global _PROGRAM
    if _PROGRAM is None:
        _PROGRAM = _build_program()
    return _PROGRAM


def _fp8(a):
    return np.clip(a, -240.0, 240.0).astype(ml_dtypes.float8_e4m3)


def _prep_inputs(x, Wq, bq, Wk, bk, Wv, bv, Wo, bo):
    """Build the 8 per-core input maps (host-side sharding, free)."""
    bf = ml_dtypes.bfloat16
    x = np.asarray(x, dtype=np.float32)
    WqT = np.ascontiguousarray(np.asarray(Wq, np.float32).T)  # [D, D] (d, f)
    WkT = np.ascontiguousarray(np.asarray(Wk, np.float32).T)
    WvT = np.ascontiguousarray(np.asarray(Wv, np.float32).T)
    WoT = np.ascontiguousarray(np.asarray(Wo, np.float32).T)  # [D, D] (f, d)

    in_maps = []
    for c in range(NCORES):
        b, g = divmod(c, GROUPS)
        fsl = slice(g * F, (g + 1) * F)
        xT = np.ascontiguousarray(x[b].T)                       # [D, T] f32
        # fp8 weights pre-packed per head: [HG, P, DC*DH], contiguous
        # per-partition lines for full-rate DMA
        def packw8(WT):
            w = _fp8(WSCALE * WT[:, fsl])                       # [D, F]
            w = w.reshape(DC, P, HG, DH).transpose(2, 1, 0, 3)  # [HG, P, DC, DH]
            return np.ascontiguousarray(w.reshape(HG, P, DC * DH))
        # wo packed [DC, P, HG, P]: partition = f-within-head chunk
        wo = np.asarray(WoT[fsl, :], np.float32).reshape(HG, P, DC, P)
        wo = np.ascontiguousarray(wo.transpose(2, 1, 0, 3)).astype(bf)
        m = {
            "x8": np.ascontiguousarray(_fp8(xT).reshape(DC, P, T)),
            "xT": np.ascontiguousarray(xT.astype(bf).reshape(DC, P, T)),
            "wq8": packw8(WqT),
            "wk8": packw8(WkT),
            "wv": np.ascontiguousarray(WvT[:, fsl]).astype(bf).reshape(DC, P, F),
            "wo": wo,
            "bq": np.ascontiguousarray(
                WSCALE * np.asarray(bq, np.float32)[fsl].reshape(HG, P).T),
            "bk": np.ascontiguousarray(
                WSCALE * np.asarray(bk, np.float32)[fsl].reshape(HG, P).T),
        }
        in_maps.append(m)
    return in_maps


def _combine(results, bv, Wo, bo):
    # softmax rows sum to 1, so the v-bias contributes bv @ Wo.T to every row
    bo_eff = (np.asarray(bo, np.float64)
              + np.asarray(Wo, np.float64) @ np.asarray(bv, np.float64)
              ).astype(np.float32)
    out = np.empty((B, T, D), dtype=np.float32)
    for b in range(B):
        oT = (results[b * GROUPS]["out"].reshape(D, T).astype(np.float32)
              + results[b * GROUPS + 1]["out"].reshape(D, T).astype(np.float32))
        out[b] = oT.T + bo_eff[None, :]
    return out


def kernel(x, Wq, bq, Wk, bk, Wv, bv, Wo, bo):
    from concourse.bass_utils import run_bass_kernel_spmd

    nc = _get_program()
    in_maps = _prep_inputs(x, Wq, bq, Wk, bk, Wv, bv, Wo, bo)
    res = run_bass_kernel_spmd(nc, in_maps, list(range(NCORES))).results
    return _combine(res, bv, Wo, bo)


# revision 40
# speedup vs baseline: 1.0153x; 1.0153x over previous
"""Trainium2 Bass kernel: multi-head attention (B=4, T=2048, D=2048, H=16).

Sharding: 8 cores = 4 batches x 2 head-groups (tensor-parallel heads, data-
parallel batch). Each core handles one batch and 8 heads (f-slice of 1024
columns of the QKV projections / rows of the out-projection). Host sums the
two partial out-projection results per batch and adds the output bias
(with bv folded in: softmax rows sum to 1, so the v-bias contributes the
constant bv @ Wo.T to every output row).

Per-core pipeline:
  Phase V: v[t,f] via x-stationary bf16 matmuls, two half-F subpasses.
  Phase QKA (per head): q/k projections in fp8 e4m3 DoubleRow mode (2
           k-tiles per matmul = 256-deep contraction at 2x PE rate; W
           pre-scaled x256 on host to avoid e4m3 subnormals, exp scale
           divided by 65536 to compensate -- softmax is scale-invariant).
           Attention per 512-wide q-stage: S^T[k,q] on PE, exp on ScalarE
           (stage rate-limiter), PV transposed (v chunks stationary, P^T
           moving, per-kc deps let PE trail the exp stream); softmax
           denominator via DVE bf16 pair-add tree (16 chunks -> 2) + one
           accumulated all-ones matmul (partition broadcast), then
           reciprocal_approx_fast and a single DVE multiply.
  Phase C: out^T[d,t] = WoT.T @ yT accumulated over f-chunks; stored
           transposed, host transposes back.
"""

import sys

if "/opt/trn_rl_repo" not in sys.path:
    sys.path.insert(0, "/opt/trn_rl_repo")

import numpy as np
import ml_dtypes

D = 2048          # d_model
T = 2048          # sequence length
B = 4             # batch
H = 16            # total heads
DH = 128          # head dim
GROUPS = 2        # head groups (tensor-parallel factor per batch)
HG = H // GROUPS  # heads per core = 8
F = HG * DH       # per-core projection width = 1024
P = 128
DC = D // P       # 16 contraction chunks
TC = T // P       # 16 t chunks
NCORES = 8
QS = 512          # attention stage width (q columns per stage)
NST = T // QS     # 4 stages per head
WSCALE = 256.0    # host pre-scale on Wq/Wk before e4m3 quantization
SCALE = float(1.0 / np.sqrt(DH))
EXP_SCALE = SCALE / (WSCALE * WSCALE)

_PROGRAM = None


def _build_program():
    import concourse.bass as bass
    import concourse.tile as tile
    from concourse import bacc, mybir
    from concourse.bass import ts, ds

    bf16 = mybir.dt.bfloat16
    f32 = mybir.dt.float32
    f8 = mybir.dt.float8e4

    nc = bacc.Bacc("TRN2", target_bir_lowering=False, debug=False,
                   num_devices=NCORES)

    x8_d = nc.dram_tensor("x8", [DC, P, T], f8, kind="ExternalInput")
    xT_d = nc.dram_tensor("xT", [DC, P, T], bf16, kind="ExternalInput")
    wq8_d = nc.dram_tensor("wq8", [HG, P, DC * DH], f8, kind="ExternalInput")
    wk8_d = nc.dram_tensor("wk8", [HG, P, DC * DH], f8, kind="ExternalInput")
    wv_d = nc.dram_tensor("wv", [DC, P, F], bf16, kind="ExternalInput")
    wo_d = nc.dram_tensor("wo", [DC, P, HG, P], bf16, kind="ExternalInput")
    bq_d = nc.dram_tensor("bq", [P, HG], f32, kind="ExternalInput")
    bk_d = nc.dram_tensor("bk", [P, HG], f32, kind="ExternalInput")
    out_d = nc.dram_tensor("out", [DC, P, T], f32, kind="ExternalOutput")

    Exp = mybir.ActivationFunctionType.Exp
    DR = mybir.MatmulPerfMode.DoubleRow
    MULT = mybir.AluOpType.mult

    with tile.TileContext(nc) as tc:
        from contextlib import ExitStack
        with ExitStack() as ctx:
            # ---- persistent pools (live whole kernel) ----
            const = ctx.enter_context(tc.tile_pool(name="const", bufs=1))
            vpool = ctx.enter_context(tc.tile_pool(name="vpool", bufs=1))
            ytp = ctx.enter_context(tc.tile_pool(name="ytp", bufs=1))
            x8p = ctx.enter_context(tc.tile_pool(name="x8p", bufs=1))
            w8p = ctx.enter_context(tc.tile_pool(name="w8p", bufs=2))
            qkt = ctx.enter_context(tc.tile_pool(name="qkt", bufs=2))
            wop = ctx.enter_context(tc.tile_pool(name="wop", bufs=4))
            ps_qk = ctx.enter_context(
                tc.tile_pool(name="ps_qk", bufs=1, space="PSUM"))

            zero_b = const.tile([P, 1], f32, tag="zerob")
            nc.vector.memset(zero_b[:], 0.0)
            ones_sb = const.tile([P, P], bf16, tag="ones")
            nc.vector.memset(ones_sb[:], 1.0)
            bq_sb = const.tile([P, HG], f32, tag="bq")
            bk_sb = const.tile([P, HG], f32, tag="bk")

            v_sb = vpool.tile([P, TC, HG, DH], bf16, tag="v")
            yT = ytp.tile([P, HG, T], bf16, tag="yT")
            x8 = x8p.tile([P, DC, T], f8, tag="x8")

            # force early allocation of persistent pools (first-use order)
            nc.vector.memset(v_sb[:, 0, 0, 0:1], 0.0)
            nc.vector.memset(yT[:, 0, 0:1], 0.0)

            w8q = [w8p.tile([P, DC, DH], f8, tag="w8q", name=f"w8q{h}")
                   for h in range(HG)]
            w8k = [w8p.tile([P, DC, DH], f8, tag="w8k", name=f"w8k{h}")
                   for h in range(HG)]

            def dma_w8(h):
                nc.sync.dma_start(
                    w8q[h][:],
                    wq8_d[h].rearrange("p (c f) -> p c f", f=DH))
                nc.sync.dma_start(
                    w8k[h][:],
                    wk8_d[h].rearrange("p (c f) -> p c f", f=DH))

            qts = {}

            def pg_open(h, kind, tb):
                if kind == "k":
                    w8h, bias_sb = w8k[h], bk_sb
                    if tb == 0:
                        qts.setdefault(h, {})["kT"] = qkt.tile(
                            [P, T], bf16, tag="kT", name=f"kT{h}")
                    dst = qts[h]["kT"]
                else:
                    w8h, bias_sb = w8q[h], bq_sb
                    if tb == 0:
                        qts.setdefault(h, {})["qT"] = qkt.tile(
                            [P, T], bf16, tag="qT", name=f"qT{h}")
                    dst = qts[h]["qT"]
                ps = ps_qk.tile([P, QS], f32, tag="psqk",
                                name=f"ps{kind}{h}_{tb}")
                return [h, kind, tb, w8h, bias_sb, dst, ps, 0]

            def pg_step(pg):
                h, kind, tb, w8h, bias_sb, dst, ps, d = pg
                nc.tensor.matmul(
                    ps[:],
                    w8h[:, ds(2 * d, 2), :],
                    x8[:, ds(2 * d, 2), ds(tb * QS, QS)],
                    start=(d == 0), stop=(d == DC // 2 - 1),
                    perf_mode=DR)
                pg[7] = d + 1
                if pg[7] == DC // 2:
                    nc.vector.tensor_scalar_add(
                        dst[:, ds(tb * QS, QS)], ps[:],
                        bias_sb[:, ds(h, 1)])
                    return None
                return pg

            def proj_group(h, kind, tb):
                pg = pg_open(h, kind, tb)
                while pg is not None:
                    pg = pg_step(pg)

            prologue = [(0, "k", 0), (0, "k", 1), (0, "k", 2), (0, "k", 3),
                        (0, "q", 0)]

            # ---------------- Phase V: v projection ----------------
            with tc.tile_pool(name="wvp", bufs=1) as wvp, \
                 tc.tile_pool(name="xbv", bufs=3) as xbv, \
                 tc.tile_pool(name="ps_v", bufs=6, space="PSUM") as ps_v:
                for half in range(2):
                    f0 = half * 512
                    wvh = wvp.tile([P, DC, 512], bf16, tag="wvh",
                                   name=f"wvh{half}")
                    xbs = [xbv.tile([P, DC, 512], bf16, tag="xbv",
                                    name=f"xbv{half}_{tcb}")
                           for tcb in range(4)]
                    if half == 0:
                        # single-dc first pair so dc 0 lands fast
                        nc.sync.dma_start(
                            wvh[:, ds(0, 1)],
                            wv_d[ds(0, 1), :, ds(f0, 512)].rearrange(
                                "c p f -> p c f"))
                        nc.sync.dma_start(
                            xbs[0][:, ds(0, 1)],
                            xT_d[ds(0, 1), :, ds(0, 512)].rearrange(
                                "c p t -> p c t"))
                        nc.sync.dma_start(
                            wvh[:, ds(1, 3)],
                            wv_d[ds(1, 3), :, ds(f0, 512)].rearrange(
                                "c p f -> p c f"))
                        nc.sync.dma_start(
                            xbs[0][:, ds(1, 3)],
                            xT_d[ds(1, 3), :, ds(0, 512)].rearrange(
                                "c p t -> p c t"))
                        for k in range(1, 4):
                            nc.sync.dma_start(
                                wvh[:, ds(4 * k, 4)],
                                wv_d[ds(4 * k, 4), :, ds(f0, 512)].rearrange(
                                    "c p f -> p c f"))
                            nc.sync.dma_start(
                                xbs[0][:, ds(4 * k, 4)],
                                xT_d[ds(4 * k, 4), :, ds(0, 512)].rearrange(
                                    "c p t -> p c t"))
                        for tcb in range(1, 4):
                            nc.sync.dma_start(
                                xbs[tcb][:],
                                xT_d[:, :, ds(tcb * 512, 512)].rearrange(
                                    "c p t -> p c t"))
                        # x8 + head-0/1 fp8 weights, behind the v-pass loads
                        nc.sync.dma_start(bq_sb[:], bq_d[:])
                        nc.sync.dma_start(bk_sb[:], bk_d[:])
                        nc.sync.dma_start(
                            x8[:], x8_d.rearrange("c p t -> p c t"))
                        dma_w8(0)
                        dma_w8(1)
                    else:
                        for k in range(4):
                            nc.sync.dma_start(
                                wvh[:, ds(4 * k, 4)],
                                wv_d[ds(4 * k, 4), :, ds(f0, 512)].rearrange(
                                    "c p f -> p c f"))
                        for tcb in range(4):
                            nc.sync.dma_start(
                                xbs[tcb][:],
                                xT_d[:, :, ds(tcb * 512, 512)].rearrange(
                                    "c p t -> p c t"))
                    for tcb in range(4):
                        xb = xbs[tcb]
                        for tsub in range(4):
                            tc_ = tcb * 4 + tsub
                            ps = ps_v.tile([P, 512], f32, tag="psv",
                                           name=f"psv{half}_{tc_}")
                            for dc in range(DC):
                                nc.tensor.matmul(
                                    ps[:],
                                    xb[:, dc, ds(tsub * P, P)],
                                    wvh[:, dc],
                                    start=(dc == 0), stop=(dc == DC - 1))
                            nc.vector.tensor_copy(
                                v_sb[:, tc_, ds(half * 4, 4), :],
                                ps[:].rearrange("p (h d) -> p h d", d=DH))
                            if half == 1 and tcb >= 2 and prologue:
                                proj_group(*prologue.pop(0))

            # ---------------- Phase QKA: q/k proj + attention ----------
            with tc.tile_pool(name="ptp", bufs=3) as ptp, \
                 tc.tile_pool(name="t8p", bufs=1) as t8p, \
                 tc.tile_pool(name="t4p", bufs=1) as t4p, \
                 tc.tile_pool(name="t2p", bufs=1) as t2p, \
                 tc.tile_pool(name="t1p", bufs=1) as t1p, \
                 tc.tile_pool(name="rinvp", bufs=2) as rinvp, \
                 tc.tile_pool(name="ps_st", bufs=5, space="PSUM") as ps_st, \
                 tc.tile_pool(name="ps_pv", bufs=1, space="PSUM") as ps_pv, \
                 tc.tile_pool(name="ps_bc", bufs=1, space="PSUM") as ps_bc:

                def finish(rec):
                    # softmax denominator broadcast + normalize (deferred
                    # one stage so the PE never waits on the DVE add-tree)
                    h, s, yps, t1 = rec
                    bc = ps_bc.tile([P, QS], f32, tag="bc", name=f"bc{h}_{s}")
                    nc.tensor.matmul(bc[:], ones_sb[:], t1[:],
                                     start=True, stop=True)
                    rinv = rinvp.tile([P, QS], f32, tag="rinv",
                                      name=f"rinv{h}_{s}")
                    nc.vector.reciprocal_approx_fast(rinv[:], bc[:])
                    nc.vector.tensor_tensor(
                        yT[:, h, ds(s * QS, QS)], yps[:], rinv[:], MULT)

                while prologue:
                    proj_group(*prologue.pop(0))
                workq = []
                cur_pg = None
                prev = None  # previous stage: (h, s, pt_t, yps, t1)
                wo_tiles = {}

                def wo_fetch(dch):
                    wo_tiles[dch] = wop.tile([P, HG, P], bf16, tag="wo",
                                             name=f"wo{dch}")
                    nc.sync.dma_start(wo_tiles[dch][:], wo_d[dch])

                for g in range(HG * NST):
                    h, s = divmod(g, NST)
                    if g >= HG * NST - 4:
                        wo_fetch(g - (HG * NST - 4))
                    if s == 0:
                        if h + 2 < HG:
                            dma_w8(h + 2)
                        for tb in range(1, NST):
                            workq.append((h, "q", tb))
                        if h + 1 < HG:
                            for tb in range(NST):
                                workq.append((h + 1, "k", tb))
                            workq.append((h + 1, "q", 0))
                    kT, qT = qts[h]["kT"], qts[h]["qT"]
                    # S^T[k,q] per k-chunk (exp -> P^T chunk), with the
                    # PREVIOUS stage's PV matmuls woven in chunk-by-chunk:
                    # they are always ready, so the PE never drains while
                    # ScalarE grinds through this stage's exp stream
                    pt_t = ptp.tile([P, TC, QS], bf16, tag="pt",
                                    name=f"pt{h}_{s}")
                    yps_prev = None
                    for kc in range(TC):
                        st = ps_st.tile([P, QS], f32, tag="st",
                                        name=f"st{h}{s}{kc}")
                        nc.tensor.matmul(st[:], kT[:, ds(kc * P, P)],
                                         qT[:, ds(s * QS, QS)],
                                         start=True, stop=True)
                        nc.scalar.activation(pt_t[:, kc], st[:], Exp,
                                             bias=0.0, scale=EXP_SCALE)
                        # previous stage's PV woven in chunk-by-chunk: its
                        # operands are always ready, so the PE never drains
                        # while ScalarE grinds through this stage's exps
                        if prev is not None:
                            if yps_prev is None:
                                yps_prev = ps_pv.tile(
                                    [P, QS], f32, tag="yps",
                                    name=f"yps{prev[0]}_{prev[1]}")
                            nc.tensor.matmul(yps_prev[:],
                                             v_sb[:, kc, prev[0]],
                                             prev[2][:, kc],
                                             start=(kc == 0),
                                             stop=(kc == TC - 1))
                    # rowsum: bf16 pair-add tree 16 -> 1 chunk on DVE
                    t8 = t8p.tile([P, 8, QS], bf16, tag="t8",
                                  name=f"t8_{h}_{s}")
                    for i in range(8):
                        nc.vector.tensor_add(t8[:, i], pt_t[:, 2 * i],
                                             pt_t[:, 2 * i + 1])
                    t4 = t4p.tile([P, 4, QS], bf16, tag="t4",
                                  name=f"t4_{h}_{s}")
                    for i in range(4):
                        nc.vector.tensor_add(t4[:, i], t8[:, 2 * i],
                                             t8[:, 2 * i + 1])
                    t2 = t2p.tile([P, 2, QS], bf16, tag="t2",
                                  name=f"t2_{h}_{s}")
                    for i in range(2):
                        nc.vector.tensor_add(t2[:, i], t4[:, 2 * i],
                                             t4[:, 2 * i + 1])
                    t1 = t1p.tile([P, QS], bf16, tag="t1",
                                  name=f"t1_{h}_{s}")
                    nc.vector.tensor_add(t1[:], t2[:, 0], t2[:, 1])
                    if prev is not None:
                        finish((prev[0], prev[1], yps_prev, prev[4]))
                    # projection groups for upcoming stages: anti-phase
                    # block that fills PE while this stage's exps drain
                    for _ in range(2):
                        if workq:
                            proj_group(*workq.pop(0))
                    prev = (h, s, pt_t, None, t1)
                # epilogue: PV + normalize for the final stage
                h, s, pt_t, _, t1 = prev
                yps = ps_pv.tile([P, QS], f32, tag="yps", name=f"yps{h}_{s}")
                for kc in range(TC):
                    nc.tensor.matmul(yps[:], v_sb[:, kc, h], pt_t[:, kc],
                                     start=(kc == 0), stop=(kc == TC - 1))
                finish((h, s, yps, t1))

            # ---------------- Phase C: out-projection ----------------
            with tc.tile_pool(name="osb", bufs=8) as osb, \
                 tc.tile_pool(name="ps_o", bufs=6, space="PSUM") as ps_o:
                for dch in range(DC):
                    if dch in wo_tiles:
                        wo_t = wo_tiles[dch]
                    else:
                        wo_t = wop.tile([P, HG, P], bf16, tag="wo",
                                        name=f"wo{dch}")
                        nc.sync.dma_start(wo_t[:], wo_d[dch])
                    pso = [ps_o.tile([P, 512], f32, tag="pso",
                                     name=f"pso{dch}_{i}") for i in range(4)]
                    if dch < DC - 1:
                        for fc in range(HG):
                            for tcb in range(4):
                                nc.tensor.matmul(
                                    pso[tcb][:],
                                    wo_t[:, fc],
                                    yT[:, fc, ds(tcb * 512, 512)],
                                    start=(fc == 0), stop=(fc == HG - 1))
                        for tcb in range(4):
                            ot = osb.tile([P, 512], f32, tag="ot",
                                          name=f"ot{dch}_{tcb}")
                            nc.vector.tensor_copy(ot[:], pso[tcb][:])
                            nc.sync.dma_start(
                                out_d[dch, :, ds(tcb * 512, 512)], ot[:])
                    else:
                        # last dch tcb-major: each psum finishes separately
                        # so 3 of 4 copy+DMA chains overlap remaining matmuls
                        for tcb in range(4):
                            for fc in range(HG):
                                nc.tensor.matmul(
                                    pso[tcb][:],
                                    wo_t[:, fc],
                                    yT[:, fc, ds(tcb * 512, 512)],
                                    start=(fc == 0), stop=(fc == HG - 1))
                            ot = osb.tile([P, 512], f32, tag="ot",
                                          name=f"ot{dch}_{tcb}")
                            nc.vector.tensor_copy(ot[:], pso[tcb][:])
                            nc.sync.dma_start(
                                out_d[dch, :, ds(tcb * 512, 512)], ot[:])

    nc.compile()
    return nc


def _get_program():
    global _PROGRAM
    if _PROGRAM is None:
        _PROGRAM = _build_program()
    return _PROGRAM


def _fp8(a):
    return np.clip(a, -240.0, 240.0).astype(ml_dtypes.float8_e4m3)


def _prep_inputs(x, Wq, bq, Wk, bk, Wv, bv, Wo, bo):
    """Build the 8 per-core input maps (host-side sharding, free)."""
    bf = ml_dtypes.bfloat16
    x = np.asarray(x, dtype=np.float32)
    WqT = np.ascontiguousarray(np.asarray(Wq, np.float32).T)  # [D, D] (d, f)
    WkT = np.ascontiguousarray(np.asarray(Wk, np.float32).T)
    WvT = np.ascontiguousarray(np.asarray(Wv, np.float32).T)
    WoT = np.ascontiguousarray(np.asarray(Wo, np.float32).T)  # [D, D] (f, d)

    in_maps = []
    for c in range(NCORES):
        b, g = divmod(c, GROUPS)
        fsl = slice(g * F, (g + 1) * F)
        xT = np.ascontiguousarray(x[b].T)                       # [D, T] f32
        # fp8 weights pre-packed per head: [HG, P, DC*DH], contiguous
        # per-partition lines for full-rate DMA
        def packw8(WT):
            w = _fp8(WSCALE * WT[:, fsl])                       # [D, F]
            w = w.reshape(DC, P, HG, DH).transpose(2, 1, 0, 3)  # [HG, P, DC, DH]
            return np.ascontiguousarray(w.reshape(HG, P, DC * DH))
        # wo packed [DC, P, HG, P]: partition = f-within-head chunk
        wo = np.asarray(WoT[fsl, :], np.float32).reshape(HG, P, DC, P)
        wo = np.ascontiguousarray(wo.transpose(2, 1, 0, 3)).astype(bf)
        m = {
            "x8": np.ascontiguousarray(_fp8(xT).reshape(DC, P, T)),
            "xT": np.ascontiguousarray(xT.astype(bf).reshape(DC, P, T)),
            "wq8": packw8(WqT),
            "wk8": packw8(WkT),
            "wv": np.ascontiguousarray(WvT[:, fsl]).astype(bf).reshape(DC, P, F),
            "wo": wo,
            "bq": np.ascontiguousarray(
                WSCALE * np.asarray(bq, np.float32)[fsl].reshape(HG, P).T),
            "bk": np.ascontiguousarray(
                WSCALE * np.asarray(bk, np.float32)[fsl].reshape(HG, P).T),
        }
        in_maps.append(m)
    return in_maps


def _combine(results, bv, Wo, bo):
    # softmax rows sum to 1, so the v-bias contributes bv @ Wo.T to every row
    bo_eff = (np.asarray(bo, np.float64)
              + np.asarray(Wo, np.float64) @ np.asarray(bv, np.float64)
              ).astype(np.float32)
    out = np.empty((B, T, D), dtype=np.float32)
    for b in range(B):
        oT = (results[b * GROUPS]["out"].reshape(D, T).astype(np.float32)
              + results[b * GROUPS + 1]["out"].reshape(D, T).astype(np.float32))
        out[b] = oT.T + bo_eff[None, :]
    return out


def kernel(x, Wq, bq, Wk, bk, Wv, bv, Wo, bo):
    from concourse.bass_utils import run_bass_kernel_spmd

    nc = _get_program()
    in_maps = _prep_inputs(x, Wq, bq, Wk, bk, Wv, bv, Wo, bo)
    res = run_bass_kernel_spmd(nc, in_maps, list(range(NCORES))).results
    return _combine(res, bv, Wo, bo)
